# revision 17
# baseline (speedup 1.0000x reference)
"""HalfKA NNUE forward pass on 8 Trainium2 NeuronCores — sparse gather version.

Network (fp32 reference):
    h1  = relu(x @ W1.T + b1)     x:[2048, 98304] sparse 0/1 (~32 nnz/row), W1:[256, 98304]
    h2  = relu(h1 @ W2.T + b2)    W2:[32, 256]
    out = h2 @ Wout.T + bout      Wout:[1, 32]  -> [2048, 1]

Strategy: data-parallel over the batch; each core handles 256 rows. Instead
of streaming the dense x (100 MB/core), the host extracts the active-feature
indices; the device gathers only the needed W1.T rows (bf16, ~4 MB/core) with
gpsimd.dma_gather and contracts them against a host-built 0/1 selection
matrix xc on the PE:

    h1.T[d, b] = sum_u W1T[U[u], d] * xc[u, b]    (U = union of the core's
                                                   active features)

dma_gather uses int16 indices (max 32767 < 98304), so the union is split into
3 windows of 32768 rows with a base-offset view of the table per window. The
SWDGE ring holds 1024 descriptors, so each window is gathered in sub-calls
(<=768 idx) spread over 4 SWDGE queues whose descriptor generation runs
concurrently. Pad slots point at the window's row 0; zeros in xc kill their
contribution. fc2/fc3 are tiny. No collectives: each core writes its own 256
outputs.
"""

import sys

sys.path.insert(0, "/opt/trn_rl_repo")

from contextlib import ExitStack

import numpy as np
import ml_dtypes

import concourse.bass as bass
import concourse.tile as tile
from concourse import bacc, mybir
from concourse.bass_utils import run_bass_kernel_spmd

f32 = mybir.dt.float32
bf16 = mybir.dt.bfloat16
i16 = mybir.dt.int16

N_CORES = 8
B = 2048
IN_DIM = 98304
H1 = 256
H2 = 32
P = 128

RPC = B // N_CORES               # 256 rows per core
NWIN = 3                         # int16 index windows over IN_DIM
WIN = 32768
CAP_W = 2816                     # gathered-index capacity per window (22 slots)
# sub-call sizes per window (each <=1024 descriptors to fit the SWDGE ring,
# multiples of 128, summing to CAP_W; first call small so the PE starts early,
# last call small so the final matmul tail is short)
SUBS_W = [
    [512, 768, 768, 768],
    [768, 768, 768, 512],
    [768, 768, 768, 512],
]
SLOTS_W = CAP_W // P             # 22
T = NWIN * SLOTS_W               # 66 k-tiles
M_T = H1 // P                    # 2 h1 partition-tiles

_CACHED = {}


def _build_program(cap_w=CAP_W, subs_w=SUBS_W):
    slots_w = cap_w // P
    t_tiles = NWIN * slots_w
    for subs in subs_w:
        assert sum(subs) == cap_w and all(s % P == 0 and s <= 1024 for s in subs)

    nc = bacc.Bacc(
        "TRN2",
        target_bir_lowering=False,
        debug=False,
        num_devices=N_CORES,
        num_swdge_queues=4,
    )

    w1t = nc.dram_tensor("w1t", [IN_DIM, H1], bf16, kind="ExternalInput")
    idxs = nc.dram_tensor("idxs", [P, NWIN, cap_w // 16], i16, kind="ExternalInput")
    xc = nc.dram_tensor("xc", [P, t_tiles, RPC], bf16, kind="ExternalInput")
    b1 = nc.dram_tensor("b1", [P, M_T], f32, kind="ExternalInput")
    w2t = nc.dram_tensor("w2t", [P, M_T, H2], f32, kind="ExternalInput")
    b2 = nc.dram_tensor("b2", [H2, 1], f32, kind="ExternalInput")
    woutt = nc.dram_tensor("woutt", [H2 + 1, 1], f32, kind="ExternalInput")
    out = nc.dram_tensor("out", [1, RPC], f32, kind="ExternalOutput")

    with tile.TileContext(nc) as tc:
        with ExitStack() as ctx:
            const = ctx.enter_context(tc.tile_pool(name="const", bufs=1))
            gp = ctx.enter_context(tc.tile_pool(name="g", bufs=1))
            actp = ctx.enter_context(tc.tile_pool(name="act", bufs=2))
            smp = ctx.enter_context(tc.tile_pool(name="small", bufs=4))
            ps1 = ctx.enter_context(tc.tile_pool(name="ps1", bufs=2, space="PSUM"))
            ps2 = ctx.enter_context(tc.tile_pool(name="ps2", bufs=2, space="PSUM"))
            ps3 = ctx.enter_context(tc.tile_pool(name="ps3", bufs=2, space="PSUM"))

            # pre-create the gather num_idxs registers with no DMA deps so
            # the register MOVEs run during boot, not gated on the idx load
            size_regs = {
                s: nc.gpsimd.to_reg(s)
                for s in sorted({s for subs in subs_w for s in subs})
            }

            # dependency-free mlp-library instruction: triggers the lazy
            # gpsimd library load (~10us) during the DMA warm-up instead of
            # at the first real gather
            warm = const.tile([P, 8], f32)
            nc.gpsimd.memset(warm[0:1, :], 0.0)
            nc.gpsimd.partition_broadcast(warm[:], warm[0:1, :])

            # idx load goes first and alone on the sync DMA queue: the first
            # gather depends only on it
            idx_s = const.tile([P, NWIN, cap_w // 16], i16)
            nc.sync.dma_start(idx_s[:], idxs.ap())

            # gathers: one sub-call chain per window, rotating SWDGE queues
            gt = gp.tile([P, t_tiles, H1], bf16, name="g", tag="g")
            qn = 0
            for w in range(NWIN):
                pos = 0
                for s in subs_w[w]:
                    s0 = w * slots_w + pos // P
                    nc.gpsimd.dma_gather(
                        gt[:, s0:s0 + s // P, :],
                        w1t.ap()[w * WIN:(w + 1) * WIN, :],
                        idx_s[:, w, pos // 16:(pos + s) // 16],
                        s,
                        size_regs[s],
                        H1,
                        queue_num=qn % 4,
                    )
                    qn += 1
                    pos += s

            # xc + small constants on the scalar/vector DMA queues so they
            # don't delay the idx load the gathers wait on
            xc_s = const.tile([P, t_tiles, RPC], bf16)
            for w in range(NWIN):
                sl = slice(w * slots_w, (w + 1) * slots_w)
                nc.scalar.dma_start(xc_s[:, sl], xc.ap()[:, sl])
            b1_s = const.tile([P, M_T], f32)
            nc.scalar.dma_start(b1_s[:], b1.ap())
            w2t_s = const.tile([P, M_T, H2], f32)
            nc.scalar.dma_start(w2t_s[:], w2t.ap())
            b2_s = const.tile([H2, 1], f32)
            nc.scalar.dma_start(b2_s[:], b2.ap())
            woutt_s = const.tile([H2 + 1, 1], f32)
            nc.scalar.dma_start(woutt_s[:], woutt.ap())

            # fc1: h1T[m][d, b] = sum_t G[:, t, m-slice].T @ xc[:, t, :]
            psum_m = [
                ps1.tile([P, RPC], f32, tag=f"ps1_{m}", name=f"ps1m{m}")
                for m in range(M_T)
            ]
            for t in range(t_tiles):
                for m in range(M_T):
                    nc.tensor.matmul(
                        psum_m[m][:],
                        gt[:, t, m * P:(m + 1) * P],
                        xc_s[:, t, :],
                        start=(t == 0),
                        stop=(t == t_tiles - 1),
                    )

            # bias+relu straight out of PSUM, then fc2/fc3
            acts = []
            for m in range(M_T):
                act = actp.tile([P, RPC], f32, name=f"act{m}", tag="act")
                nc.scalar.activation(
                    act[:], psum_m[m][:],
                    mybir.ActivationFunctionType.Relu,
                    bias=b1_s[:, m:m + 1],
                )
                acts.append(act)

            p2 = ps2.tile([H2, RPC], f32, name="p2", tag="p2")
            for m in range(M_T):
                nc.tensor.matmul(
                    p2[:], w2t_s[:, m, :], acts[m][:],
                    start=(m == 0), stop=(m == M_T - 1),
                )
            h2t = smp.tile([H2 + 1, RPC], f32, tag="h2", name="h2t")
            nc.scalar.activation(
                h2t[0:H2, :], p2[:],
                mybir.ActivationFunctionType.Relu,
                bias=b2_s[:],
            )
            nc.vector.memset(h2t[H2:H2 + 1, :], 1.0)

            p3 = ps3.tile([1, RPC], f32, name="p3", tag="p3")
            nc.tensor.matmul(p3[:], woutt_s[:], h2t[:], start=True, stop=True)
            ot = smp.tile([1, RPC], f32, tag="ot", name="ot")
            nc.vector.tensor_copy(ot[:], p3[:])
            nc.sync.dma_start(out.ap()[0, :], ot[:])

    nc.compile()
    return nc


def _default_subs(cap_w):
    subs = [1024] * (cap_w // 1024)
    if cap_w % 1024:
        subs.append(cap_w % 1024)
    return subs


def get_program(cap_w=CAP_W):
    key = ("nc", cap_w)
    if key not in _CACHED:
        subs_w = SUBS_W if cap_w == CAP_W else [_default_subs(cap_w)] * NWIN
        _CACHED[key] = _build_program(cap_w, subs_w)
    return _CACHED[key]


def _pack_idxs(local, cap_w):
    """[cap_w] int16 position-ordered indices -> [P, cap_w//16] SBUF layout.

    Position i is read from partition i%16, column i//16; the 16-partition
    block is replicated across all 128 partitions.
    """
    arr = local.reshape(cap_w // 16, 16).T  # [16, cols]
    return np.tile(arr, (8, 1))             # [128, cols]


def _prep_inputs(x, W1, b1, W2, b2, Wout, bout, cap_w):
    bf = ml_dtypes.bfloat16
    slots_w = cap_w // P
    t_tiles = NWIN * slots_w

    w1t_h = np.ascontiguousarray(W1.T.astype(bf))                # [98304, 256]
    b1_h = np.ascontiguousarray(b1.reshape(M_T, P).T)            # [P, M_T]
    w2t_h = np.ascontiguousarray(W2.T.reshape(M_T, P, H2).transpose(1, 0, 2))
    b2_h = np.ascontiguousarray(b2.reshape(H2, 1))
    woutt_h = np.concatenate(
        [Wout.T, bout.reshape(1, 1)], axis=0
    ).astype(np.float32)                                         # [H2+1, 1]

    rows_all, cols_all = np.nonzero(x != 0.0)

    in_maps = []
    for c in range(N_CORES):
        lo = c * RPC
        sel = (rows_all >= lo) & (rows_all < lo + RPC)
        bs = (rows_all[sel] - lo).astype(np.int64)
        fs = cols_all[sel].astype(np.int64)
        posmap = np.full(IN_DIM, -1, dtype=np.int64)
        u_all = np.unique(fs)
        idx_h = np.zeros((P, NWIN, cap_w // 16), dtype=np.int16)
        for w in range(NWIN):
            uw = u_all[(u_all >= w * WIN) & (u_all < (w + 1) * WIN)]
            n_w = len(uw)
            if n_w > cap_w:
                raise OverflowError(n_w)
            local = np.zeros(cap_w, dtype=np.int16)
            local[:n_w] = (uw - w * WIN).astype(np.int16)
            idx_h[:, w, :] = _pack_idxs(local, cap_w)
            j = np.arange(n_w)
            posmap[uw] = (w * slots_w + j // P) * P + (j % P)
        xc_h = np.zeros((t_tiles * P, RPC), dtype=np.float32)
        xc_h[posmap[fs], bs] = 1.0
        in_maps.append({
            "w1t": w1t_h,
            "idxs": idx_h,
            "xc": np.ascontiguousarray(
                xc_h.reshape(t_tiles, P, RPC).transpose(1, 0, 2).astype(bf)
            ),
            "b1": b1_h,
            "w2t": w2t_h,
            "b2": b2_h,
            "woutt": woutt_h,
        })
    return in_maps


def kernel(x, W1, b1, W2, b2, Wout, bout, _trace=False, _trace_kwargs=None):
    x = np.asarray(x, dtype=np.float32)
    W1 = np.asarray(W1, dtype=np.float32)
    b1 = np.asarray(b1, dtype=np.float32)
    W2 = np.asarray(W2, dtype=np.float32)
    b2 = np.asarray(b2, dtype=np.float32)
    Wout = np.asarray(Wout, dtype=np.float32)
    bout = np.asarray(bout, dtype=np.float32)

    cap_w = CAP_W
    while True:
        try:
            in_maps = _prep_inputs(x, W1, b1, W2, b2, Wout, bout, cap_w)
            break
        except OverflowError as e:
            # denser input than expected: grow the per-window capacity
            cap_w = ((int(e.args[0]) + P - 1) // P + 1) * P

    nc = get_program(cap_w)
    res = run_bass_kernel_spmd(
        nc,
        in_maps,
        core_ids=list(range(N_CORES)),
        trace=_trace,
        **(_trace_kwargs or {}),
    )
    out = np.concatenate(
        [res.results[c]["out"].reshape(RPC) for c in range(N_CORES)]
    ).reshape(B, 1).astype(np.float32)
    if _trace:
        kernel.last_results = res
    return out


if __name__ == "__main__":
    rng = np.random.default_rng(0)
    x = (rng.random((B, IN_DIM)) < 32.0 / IN_DIM).astype(np.float32)
    W1 = rng.standard_normal((H1, IN_DIM), dtype=np.float32) / np.sqrt(IN_DIM)
    b1 = rng.standard_normal(H1, dtype=np.float32) / np.sqrt(IN_DIM)
    W2 = rng.standard_normal((H2, H1), dtype=np.float32) / np.sqrt(H1)
    b2 = rng.standard_normal(H2, dtype=np.float32) / np.sqrt(H1)
    Wout = rng.standard_normal((1, H2), dtype=np.float32) / np.sqrt(H2)
    bout = rng.standard_normal(1, dtype=np.float32) / np.sqrt(H2)
    got = kernel(x, W1, b1, W2, b2, Wout, bout)
    h1 = np.maximum(x @ W1.T + b1, 0)
    h2 = np.maximum(h1 @ W2.T + b2, 0)
    exp = h2 @ Wout.T + bout
    print("rel err:", np.abs(got - exp).max() / np.abs(exp).max())


# revision 20
# speedup vs baseline: 1.0722x; 1.0722x over previous
"""HalfKA NNUE forward pass on 8 Trainium2 NeuronCores — sparse gather version.

Network (fp32 reference):
    h1  = relu(x @ W1.T + b1)     x:[2048, 98304] sparse 0/1 (~32 nnz/row), W1:[256, 98304]
    h2  = relu(h1 @ W2.T + b2)    W2:[32, 256]
    out = h2 @ Wout.T + bout      Wout:[1, 32]  -> [2048, 1]

Strategy: data-parallel over the batch; each core handles 256 rows. Instead
of streaming the dense x (100 MB/core), the host extracts the active-feature
indices; the device gathers only the needed W1.T rows (bf16, ~4 MB/core) with
gpsimd.dma_gather and contracts them against a host-built 0/1 selection
matrix xc on the PE:

    h1.T[d, b] = sum_u W1T[U[u], d] * xc[u, b]    (U = union of the core's
                                                   active features)

dma_gather uses int16 indices (max 32767 < 98304), so the union is split into
3 windows of 32768 rows with a base-offset view of the table per window. The
SWDGE ring holds 1024 descriptors, so each window is gathered in sub-calls
(<=768 idx) spread over 4 SWDGE queues whose descriptor generation runs
concurrently. Pad slots point at the window's row 0; zeros in xc kill their
contribution. fc2/fc3 are tiny. No collectives: each core writes its own 256
outputs.
"""

import sys

sys.path.insert(0, "/opt/trn_rl_repo")

from contextlib import ExitStack

import numpy as np
import ml_dtypes

import concourse.bass as bass
import concourse.tile as tile
from concourse import bacc, mybir
from concourse.bass_utils import run_bass_kernel_spmd

f32 = mybir.dt.float32
bf16 = mybir.dt.bfloat16
i16 = mybir.dt.int16

N_CORES = 8
B = 2048
IN_DIM = 98304
H1 = 256
H2 = 32
P = 128

RPC = B // N_CORES               # 256 rows per core
NWIN = 3                         # int16 index windows over IN_DIM
WIN = 32768
CAP_W = 2816                     # gathered-index capacity per window (22 slots)
# sub-call sizes per window (each <=1024 descriptors to fit the SWDGE ring,
# multiples of 128, summing to CAP_W; first call small so the PE starts early,
# last call small so the final matmul tail is short)
SUBS_W = [
    [512, 768, 768, 768],
    [768, 768, 768, 512],
    [768, 768, 768, 512],
]
SLOTS_W = CAP_W // P             # 22
T = NWIN * SLOTS_W               # 66 k-tiles
M_T = H1 // P                    # 2 h1 partition-tiles

_CACHED = {}


def _build_program(cap_w=CAP_W, subs_w=SUBS_W):
    slots_w = cap_w // P
    t_tiles = NWIN * slots_w
    for subs in subs_w:
        assert sum(subs) == cap_w and all(s % P == 0 and s <= 1024 for s in subs)

    nc = bacc.Bacc(
        "TRN2",
        target_bir_lowering=False,
        debug=False,
        num_devices=N_CORES,
        num_swdge_queues=4,
    )

    w1t = nc.dram_tensor("w1t", [IN_DIM, H1], bf16, kind="ExternalInput")
    idxs = nc.dram_tensor("idxs", [P, NWIN, cap_w // 16], i16, kind="ExternalInput")
    xc = nc.dram_tensor("xc", [P, t_tiles, RPC], bf16, kind="ExternalInput")
    b1 = nc.dram_tensor("b1", [P, M_T], f32, kind="ExternalInput")
    w2t = nc.dram_tensor("w2t", [P, M_T, H2], f32, kind="ExternalInput")
    b2 = nc.dram_tensor("b2", [H2, 1], f32, kind="ExternalInput")
    woutt = nc.dram_tensor("woutt", [H2 + 1, 1], f32, kind="ExternalInput")
    out = nc.dram_tensor("out", [1, RPC], f32, kind="ExternalOutput")

    with tile.TileContext(nc) as tc:
        with ExitStack() as ctx:
            const = ctx.enter_context(tc.tile_pool(name="const", bufs=1))
            gp = ctx.enter_context(tc.tile_pool(name="g", bufs=1))
            actp = ctx.enter_context(tc.tile_pool(name="act", bufs=2))
            smp = ctx.enter_context(tc.tile_pool(name="small", bufs=4))
            ps1 = ctx.enter_context(tc.tile_pool(name="ps1", bufs=2, space="PSUM"))
            ps2 = ctx.enter_context(tc.tile_pool(name="ps2", bufs=2, space="PSUM"))
            ps3 = ctx.enter_context(tc.tile_pool(name="ps3", bufs=2, space="PSUM"))

            # idx load goes first and alone on the sync DMA queue: the first
            # gather depends only on it
            idx_s = const.tile([P, NWIN, cap_w // 16], i16)
            nc.sync.dma_start(idx_s[:], idxs.ap())

            # gathers: sub-calls interleaved across windows on rotating SWDGE
            # queues, so G tiles arrive spread over all three windows and the
            # final call leaves only a short matmul tail
            call_order = []
            pos_w = [0] * NWIN
            for si in range(max(len(s) for s in subs_w)):
                for w in range(NWIN):
                    if si < len(subs_w[w]):
                        s = subs_w[w][si]
                        call_order.append((w, pos_w[w], s))
                        pos_w[w] += s

            gt = gp.tile([P, t_tiles, H1], bf16, name="g", tag="g")
            for qn, (w, pos, s) in enumerate(call_order):
                s0 = w * slots_w + pos // P
                nc.gpsimd.dma_gather(
                    gt[:, s0:s0 + s // P, :],
                    w1t.ap()[w * WIN:(w + 1) * WIN, :],
                    idx_s[:, w, pos // 16:(pos + s) // 16],
                    s,
                    s,
                    H1,
                    queue_num=qn % 4,
                )

            # xc + small constants on the scalar/vector DMA queues so they
            # don't delay the idx load the gathers wait on
            xc_s = const.tile([P, t_tiles, RPC], bf16)
            for w in range(NWIN):
                sl = slice(w * slots_w, (w + 1) * slots_w)
                nc.scalar.dma_start(xc_s[:, sl], xc.ap()[:, sl])
            b1_s = const.tile([P, M_T], f32)
            nc.scalar.dma_start(b1_s[:], b1.ap())
            w2t_s = const.tile([P, M_T, H2], f32)
            nc.scalar.dma_start(w2t_s[:], w2t.ap())
            b2_s = const.tile([H2, 1], f32)
            nc.scalar.dma_start(b2_s[:], b2.ap())
            woutt_s = const.tile([H2 + 1, 1], f32)
            nc.scalar.dma_start(woutt_s[:], woutt.ap())

            # fc1: h1T[m][d, b] = sum_t G[:, t, m-slice].T @ xc[:, t, :]
            # k-tiles consumed in gather-arrival order (accumulation is
            # order-free; only the first/last need start/stop)
            t_order = [
                w * slots_w + pos // P + i
                for (w, pos, s) in call_order
                for i in range(s // P)
            ]
            assert sorted(t_order) == list(range(t_tiles))
            psum_m = [
                ps1.tile([P, RPC], f32, tag=f"ps1_{m}", name=f"ps1m{m}")
                for m in range(M_T)
            ]
            for ti, t in enumerate(t_order):
                for m in range(M_T):
                    nc.tensor.matmul(
                        psum_m[m][:],
                        gt[:, t, m * P:(m + 1) * P],
                        xc_s[:, t, :],
                        start=(ti == 0),
                        stop=(ti == t_tiles - 1),
                    )

            # bias+relu straight out of PSUM, then fc2/fc3
            acts = []
            for m in range(M_T):
                act = actp.tile([P, RPC], f32, name=f"act{m}", tag="act")
                nc.scalar.activation(
                    act[:], psum_m[m][:],
                    mybir.ActivationFunctionType.Relu,
                    bias=b1_s[:, m:m + 1],
                )
                acts.append(act)

            p2 = ps2.tile([H2, RPC], f32, name="p2", tag="p2")
            for m in range(M_T):
                nc.tensor.matmul(
                    p2[:], w2t_s[:, m, :], acts[m][:],
                    start=(m == 0), stop=(m == M_T - 1),
                )
            h2t = smp.tile([H2 + 1, RPC], f32, tag="h2", name="h2t")
            nc.scalar.activation(
                h2t[0:H2, :], p2[:],
                mybir.ActivationFunctionType.Relu,
                bias=b2_s[:],
            )
            nc.vector.memset(h2t[H2:H2 + 1, :], 1.0)

            p3 = ps3.tile([1, RPC], f32, name="p3", tag="p3")
            nc.tensor.matmul(p3[:], woutt_s[:], h2t[:], start=True, stop=True)
            ot = smp.tile([1, RPC], f32, tag="ot", name="ot")
            nc.vector.tensor_copy(ot[:], p3[:])
            nc.sync.dma_start(out.ap()[0, :], ot[:])

    nc.compile()
    return nc


def _default_subs(cap_w):
    subs = [1024] * (cap_w // 1024)
    if cap_w % 1024:
        subs.append(cap_w % 1024)
    return subs


def get_program(cap_w=CAP_W):
    key = ("nc", cap_w)
    if key not in _CACHED:
        subs_w = SUBS_W if cap_w == CAP_W else [_default_subs(cap_w)] * NWIN
        _CACHED[key] = _build_program(cap_w, subs_w)
    return _CACHED[key]


def _pack_idxs(local, cap_w):
    """[cap_w] int16 position-ordered indices -> [P, cap_w//16] SBUF layout.

    Position i is read from partition i%16, column i//16; the 16-partition
    block is replicated across all 128 partitions.
    """
    arr = local.reshape(cap_w // 16, 16).T  # [16, cols]
    return np.tile(arr, (8, 1))             # [128, cols]


def _prep_inputs(x, W1, b1, W2, b2, Wout, bout, cap_w):
    bf = ml_dtypes.bfloat16
    slots_w = cap_w // P
    t_tiles = NWIN * slots_w

    w1t_h = np.ascontiguousarray(W1.T.astype(bf))                # [98304, 256]
    b1_h = np.ascontiguousarray(b1.reshape(M_T, P).T)            # [P, M_T]
    w2t_h = np.ascontiguousarray(W2.T.reshape(M_T, P, H2).transpose(1, 0, 2))
    b2_h = np.ascontiguousarray(b2.reshape(H2, 1))
    woutt_h = np.concatenate(
        [Wout.T, bout.reshape(1, 1)], axis=0
    ).astype(np.float32)                                         # [H2+1, 1]

    rows_all, cols_all = np.nonzero(x != 0.0)

    in_maps = []
    for c in range(N_CORES):
        lo = c * RPC
        sel = (rows_all >= lo) & (rows_all < lo + RPC)
        bs = (rows_all[sel] - lo).astype(np.int64)
        fs = cols_all[sel].astype(np.int64)
        posmap = np.full(IN_DIM, -1, dtype=np.int64)
        u_all = np.unique(fs)
        idx_h = np.zeros((P, NWIN, cap_w // 16), dtype=np.int16)
        for w in range(NWIN):
            uw = u_all[(u_all >= w * WIN) & (u_all < (w + 1) * WIN)]
            n_w = len(uw)
            if n_w > cap_w:
                raise OverflowError(n_w)
            local = np.zeros(cap_w, dtype=np.int16)
            local[:n_w] = (uw - w * WIN).astype(np.int16)
            idx_h[:, w, :] = _pack_idxs(local, cap_w)
            j = np.arange(n_w)
            posmap[uw] = (w * slots_w + j // P) * P + (j % P)
        xc_h = np.zeros((t_tiles * P, RPC), dtype=np.float32)
        xc_h[posmap[fs], bs] = 1.0
        in_maps.append({
            "w1t": w1t_h,
            "idxs": idx_h,
            "xc": np.ascontiguousarray(
                xc_h.reshape(t_tiles, P, RPC).transpose(1, 0, 2).astype(bf)
            ),
            "b1": b1_h,
            "w2t": w2t_h,
            "b2": b2_h,
            "woutt": woutt_h,
        })
    return in_maps


def kernel(x, W1, b1, W2, b2, Wout, bout, _trace=False, _trace_kwargs=None):
    x = np.asarray(x, dtype=np.float32)
    W1 = np.asarray(W1, dtype=np.float32)
    b1 = np.asarray(b1, dtype=np.float32)
    W2 = np.asarray(W2, dtype=np.float32)
    b2 = np.asarray(b2, dtype=np.float32)
    Wout = np.asarray(Wout, dtype=np.float32)
    bout = np.asarray(bout, dtype=np.float32)

    cap_w = CAP_W
    while True:
        try:
            in_maps = _prep_inputs(x, W1, b1, W2, b2, Wout, bout, cap_w)
            break
        except OverflowError as e:
            # denser input than expected: grow the per-window capacity
            cap_w = ((int(e.args[0]) + P - 1) // P + 1) * P

    nc = get_program(cap_w)
    res = run_bass_kernel_spmd(
        nc,
        in_maps,
        core_ids=list(range(N_CORES)),
        trace=_trace,
        **(_trace_kwargs or {}),
    )
    out = np.concatenate(
        [res.results[c]["out"].reshape(RPC) for c in range(N_CORES)]
    ).reshape(B, 1).astype(np.float32)
    if _trace:
        kernel.last_results = res
    return out


if __name__ == "__main__":
    rng = np.random.default_rng(0)
    x = (rng.random((B, IN_DIM)) < 32.0 / IN_DIM).astype(np.float32)
    W1 = rng.standard_normal((H1, IN_DIM), dtype=np.float32) / np.sqrt(IN_DIM)
    b1 = rng.standard_normal(H1, dtype=np.float32) / np.sqrt(IN_DIM)
    W2 = rng.standard_normal((H2, H1), dtype=np.float32) / np.sqrt(H1)
    b2 = rng.standard_normal(H2, dtype=np.float32) / np.sqrt(H1)
    Wout = rng.standard_normal((1, H2), dtype=np.float32) / np.sqrt(H2)
    bout = rng.standard_normal(1, dtype=np.float32) / np.sqrt(H2)
    got = kernel(x, W1, b1, W2, b2, Wout, bout)
    h1 = np.maximum(x @ W1.T + b1, 0)
    h2 = np.maximum(h1 @ W2.T + b2, 0)
    exp = h2 @ Wout.T + bout
    print("rel err:", np.abs(got - exp).max() / np.abs(exp).max())


# revision 21
# speedup vs baseline: 1.1323x; 1.0560x over previous
"""HalfKA NNUE forward pass on 8 Trainium2 NeuronCores — sparse gather version.

Network (fp32 reference):
    h1  = relu(x @ W1.T + b1)     x:[2048, 98304] sparse 0/1 (~32 nnz/row), W1:[256, 98304]
    h2  = relu(h1 @ W2.T + b2)    W2:[32, 256]
    out = h2 @ Wout.T + bout      Wout:[1, 32]  -> [2048, 1]

Strategy: data-parallel over the batch; each core handles 256 rows. Instead
of streaming the dense x (100 MB/core), the host extracts the active-feature
indices; the device gathers only the needed W1.T rows (bf16, ~4 MB/core) with
gpsimd.dma_gather and contracts them against a host-built 0/1 selection
matrix xc on the PE:

    h1.T[d, b] = sum_u W1T[U[u], d] * xc[u, b]    (U = union of the core's
                                                   active features)

dma_gather uses int16 indices (max 32767 < 98304), so the union is split into
3 windows of 32768 rows with a base-offset view of the table per window. The
SWDGE ring holds 1024 descriptors, so each window is gathered in sub-calls
(<=768 idx) spread over 4 SWDGE queues whose descriptor generation runs
concurrently. Pad slots point at the window's row 0; zeros in xc kill their
contribution. fc2/fc3 are tiny. No collectives: each core writes its own 256
outputs.
"""

import sys

sys.path.insert(0, "/opt/trn_rl_repo")

from contextlib import ExitStack

import numpy as np
import ml_dtypes

import concourse.bass as bass
import concourse.tile as tile
from concourse import bacc, mybir
from concourse.bass_utils import run_bass_kernel_spmd

f32 = mybir.dt.float32
bf16 = mybir.dt.bfloat16
i16 = mybir.dt.int16

N_CORES = 8
B = 2048
IN_DIM = 98304
H1 = 256
H2 = 32
P = 128

RPC = B // N_CORES               # 256 rows per core
NWIN = 3                         # int16 index windows over IN_DIM
WIN = 32768
CAP_W = 2816                     # gathered-index capacity per window (22 slots)
# sub-call sizes per window (each <=1024 descriptors to fit the SWDGE ring,
# multiples of 128, summing to CAP_W; first call small so the PE starts early,
# last call small so the final matmul tail is short)
SUBS_W = [
    [512, 768, 768, 768],
    [768, 768, 768, 512],
    [768, 768, 768, 512],
]
SLOTS_W = CAP_W // P             # 22
T = NWIN * SLOTS_W               # 66 k-tiles
M_T = H1 // P                    # 2 h1 partition-tiles

_CACHED = {}


def _build_program(cap_w=CAP_W, subs_w=SUBS_W):
    slots_w = cap_w // P
    t_tiles = NWIN * slots_w
    for subs in subs_w:
        assert sum(subs) == cap_w and all(s % P == 0 and s <= 1024 for s in subs)

    nc = bacc.Bacc(
        "TRN2",
        target_bir_lowering=False,
        debug=False,
        num_devices=N_CORES,
        num_swdge_queues=4,
    )

    w1t = nc.dram_tensor("w1t", [IN_DIM, H1], bf16, kind="ExternalInput")
    idxs = nc.dram_tensor("idxs", [P, NWIN, cap_w // 16], i16, kind="ExternalInput")
    xc = nc.dram_tensor("xc", [P, t_tiles, RPC], bf16, kind="ExternalInput")
    b1 = nc.dram_tensor("b1", [P, M_T], f32, kind="ExternalInput")
    w2t = nc.dram_tensor("w2t", [P, M_T, H2], f32, kind="ExternalInput")
    b2 = nc.dram_tensor("b2", [H2, 1], f32, kind="ExternalInput")
    woutt = nc.dram_tensor("woutt", [H2 + 1, 1], f32, kind="ExternalInput")
    out = nc.dram_tensor("out", [1, RPC], f32, kind="ExternalOutput")

    with tile.TileContext(nc) as tc:
        with ExitStack() as ctx:
            const = ctx.enter_context(tc.tile_pool(name="const", bufs=1))
            gp = ctx.enter_context(tc.tile_pool(name="g", bufs=1))
            actp = ctx.enter_context(tc.tile_pool(name="act", bufs=2))
            smp = ctx.enter_context(tc.tile_pool(name="small", bufs=4))
            ps1 = ctx.enter_context(tc.tile_pool(name="ps1", bufs=2, space="PSUM"))
            ps2 = ctx.enter_context(tc.tile_pool(name="ps2", bufs=2, space="PSUM"))
            ps3 = ctx.enter_context(tc.tile_pool(name="ps3", bufs=2, space="PSUM"))

            # idx load goes first and alone on the sync DMA queue: the first
            # gather depends only on it
            idx_s = const.tile([P, NWIN, cap_w // 16], i16)
            nc.sync.dma_start(idx_s[:], idxs.ap())

            # gathers: sub-calls interleaved across windows on rotating SWDGE
            # queues, so G tiles arrive spread over all three windows and the
            # final call leaves only a short matmul tail
            call_order = []
            for w in range(NWIN):
                pos = 0
                for s in subs_w[w]:
                    call_order.append((w, pos, s))
                    pos += s

            gt = gp.tile([P, t_tiles, H1], bf16, name="g", tag="g")
            for qn, (w, pos, s) in enumerate(call_order):
                s0 = w * slots_w + pos // P
                nc.gpsimd.dma_gather(
                    gt[:, s0:s0 + s // P, :],
                    w1t.ap()[w * WIN:(w + 1) * WIN, :],
                    idx_s[:, w, pos // 16:(pos + s) // 16],
                    s,
                    s,
                    H1,
                    queue_num=qn % 4,
                )

            # xc + small constants on the scalar/vector DMA queues so they
            # don't delay the idx load the gathers wait on
            xc_s = const.tile([P, t_tiles, RPC], bf16)
            for w in range(NWIN):
                sl = slice(w * slots_w, (w + 1) * slots_w)
                nc.scalar.dma_start(xc_s[:, sl], xc.ap()[:, sl])
            b1_s = const.tile([P, M_T], f32)
            nc.scalar.dma_start(b1_s[:], b1.ap())
            w2t_s = const.tile([P, M_T, H2], f32)
            nc.scalar.dma_start(w2t_s[:], w2t.ap())
            b2_s = const.tile([H2, 1], f32)
            nc.scalar.dma_start(b2_s[:], b2.ap())
            woutt_s = const.tile([H2 + 1, 1], f32)
            nc.scalar.dma_start(woutt_s[:], woutt.ap())

            # fc1: h1T[m][d, b] = sum_t G[:, t, m-slice].T @ xc[:, t, :]
            # k-tiles consumed in gather-arrival order (accumulation is
            # order-free; only the first/last need start/stop)
            t_order = [
                w * slots_w + pos // P + i
                for (w, pos, s) in call_order
                for i in range(s // P)
            ]
            assert sorted(t_order) == list(range(t_tiles))
            psum_m = [
                ps1.tile([P, RPC], f32, tag=f"ps1_{m}", name=f"ps1m{m}")
                for m in range(M_T)
            ]
            for ti, t in enumerate(t_order):
                for m in range(M_T):
                    nc.tensor.matmul(
                        psum_m[m][:],
                        gt[:, t, m * P:(m + 1) * P],
                        xc_s[:, t, :],
                        start=(ti == 0),
                        stop=(ti == t_tiles - 1),
                    )

            # bias+relu straight out of PSUM, then fc2/fc3
            acts = []
            for m in range(M_T):
                act = actp.tile([P, RPC], f32, name=f"act{m}", tag="act")
                nc.scalar.activation(
                    act[:], psum_m[m][:],
                    mybir.ActivationFunctionType.Relu,
                    bias=b1_s[:, m:m + 1],
                )
                acts.append(act)

            p2 = ps2.tile([H2, RPC], f32, name="p2", tag="p2")
            for m in range(M_T):
                nc.tensor.matmul(
                    p2[:], w2t_s[:, m, :], acts[m][:],
                    start=(m == 0), stop=(m == M_T - 1),
                )
            h2t = smp.tile([H2 + 1, RPC], f32, tag="h2", name="h2t")
            nc.scalar.activation(
                h2t[0:H2, :], p2[:],
                mybir.ActivationFunctionType.Relu,
                bias=b2_s[:],
            )
            nc.vector.memset(h2t[H2:H2 + 1, :], 1.0)

            p3 = ps3.tile([1, RPC], f32, name="p3", tag="p3")
            nc.tensor.matmul(p3[:], woutt_s[:], h2t[:], start=True, stop=True)
            ot = smp.tile([1, RPC], f32, tag="ot", name="ot")
            nc.vector.tensor_copy(ot[:], p3[:])
            nc.sync.dma_start(out.ap()[0, :], ot[:])

    nc.compile()
    return nc


def _default_subs(cap_w):
    subs = [1024] * (cap_w // 1024)
    if cap_w % 1024:
        subs.append(cap_w % 1024)
    return subs


def get_program(cap_w=CAP_W):
    key = ("nc", cap_w)
    if key not in _CACHED:
        subs_w = SUBS_W if cap_w == CAP_W else [_default_subs(cap_w)] * NWIN
        _CACHED[key] = _build_program(cap_w, subs_w)
    return _CACHED[key]


def _pack_idxs(local, cap_w):
    """[cap_w] int16 position-ordered indices -> [P, cap_w//16] SBUF layout.

    Position i is read from partition i%16, column i//16; the 16-partition
    block is replicated across all 128 partitions.
    """
    arr = local.reshape(cap_w // 16, 16).T  # [16, cols]
    return np.tile(arr, (8, 1))             # [128, cols]


def _prep_inputs(x, W1, b1, W2, b2, Wout, bout, cap_w):
    bf = ml_dtypes.bfloat16
    slots_w = cap_w // P
    t_tiles = NWIN * slots_w

    w1t_h = np.ascontiguousarray(W1.T.astype(bf))                # [98304, 256]
    b1_h = np.ascontiguousarray(b1.reshape(M_T, P).T)            # [P, M_T]
    w2t_h = np.ascontiguousarray(W2.T.reshape(M_T, P, H2).transpose(1, 0, 2))
    b2_h = np.ascontiguousarray(b2.reshape(H2, 1))
    woutt_h = np.concatenate(
        [Wout.T, bout.reshape(1, 1)], axis=0
    ).astype(np.float32)                                         # [H2+1, 1]

    rows_all, cols_all = np.nonzero(x != 0.0)

    in_maps = []
    for c in range(N_CORES):
        lo = c * RPC
        sel = (rows_all >= lo) & (rows_all < lo + RPC)
        bs = (rows_all[sel] - lo).astype(np.int64)
        fs = cols_all[sel].astype(np.int64)
        posmap = np.full(IN_DIM, -1, dtype=np.int64)
        u_all = np.unique(fs)
        idx_h = np.zeros((P, NWIN, cap_w // 16), dtype=np.int16)
        for w in range(NWIN):
            uw = u_all[(u_all >= w * WIN) & (u_all < (w + 1) * WIN)]
            n_w = len(uw)
            if n_w > cap_w:
                raise OverflowError(n_w)
            local = np.zeros(cap_w, dtype=np.int16)
            local[:n_w] = (uw - w * WIN).astype(np.int16)
            idx_h[:, w, :] = _pack_idxs(local, cap_w)
            j = np.arange(n_w)
            posmap[uw] = (w * slots_w + j // P) * P + (j % P)
        xc_h = np.zeros((t_tiles * P, RPC), dtype=np.float32)
        xc_h[posmap[fs], bs] = 1.0
        in_maps.append({
            "w1t": w1t_h,
            "idxs": idx_h,
            "xc": np.ascontiguousarray(
                xc_h.reshape(t_tiles, P, RPC).transpose(1, 0, 2).astype(bf)
            ),
            "b1": b1_h,
            "w2t": w2t_h,
            "b2": b2_h,
            "woutt": woutt_h,
        })
    return in_maps


def kernel(x, W1, b1, W2, b2, Wout, bout, _trace=False, _trace_kwargs=None):
    x = np.asarray(x, dtype=np.float32)
    W1 = np.asarray(W1, dtype=np.float32)
    b1 = np.asarray(b1, dtype=np.float32)
    W2 = np.asarray(W2, dtype=np.float32)
    b2 = np.asarray(b2, dtype=np.float32)
    Wout = np.asarray(Wout, dtype=np.float32)
    bout = np.asarray(bout, dtype=np.float32)

    cap_w = CAP_W
    while True:
        try:
            in_maps = _prep_inputs(x, W1, b1, W2, b2, Wout, bout, cap_w)
            break
        except OverflowError as e:
            # denser input than expected: grow the per-window capacity
            cap_w = ((int(e.args[0]) + P - 1) // P + 1) * P

    nc = get_program(cap_w)
    res = run_bass_kernel_spmd(
        nc,
        in_maps,
        core_ids=list(range(N_CORES)),
        trace=_trace,
        **(_trace_kwargs or {}),
    )
    out = np.concatenate(
        [res.results[c]["out"].reshape(RPC) for c in range(N_CORES)]
    ).reshape(B, 1).astype(np.float32)
    if _trace:
        kernel.last_results = res
    return out


if __name__ == "__main__":
    rng = np.random.default_rng(0)
    x = (rng.random((B, IN_DIM)) < 32.0 / IN_DIM).astype(np.float32)
    W1 = rng.standard_normal((H1, IN_DIM), dtype=np.float32) / np.sqrt(IN_DIM)
    b1 = rng.standard_normal(H1, dtype=np.float32) / np.sqrt(IN_DIM)
    W2 = rng.standard_normal((H2, H1), dtype=np.float32) / np.sqrt(H1)
    b2 = rng.standard_normal(H2, dtype=np.float32) / np.sqrt(H1)
    Wout = rng.standard_normal((1, H2), dtype=np.float32) / np.sqrt(H2)
    bout = rng.standard_normal(1, dtype=np.float32) / np.sqrt(H2)
    got = kernel(x, W1, b1, W2, b2, Wout, bout)
    h1 = np.maximum(x @ W1.T + b1, 0)
    h2 = np.maximum(h1 @ W2.T + b2, 0)
    exp = h2 @ Wout.T + bout
    print("rel err:", np.abs(got - exp).max() / np.abs(exp).max())


# revision 24
# speedup vs baseline: 1.1470x; 1.0130x over previous
"""HalfKA NNUE forward pass on 8 Trainium2 NeuronCores — sparse gather version.

Network (fp32 reference):
    h1  = relu(x @ W1.T + b1)     x:[2048, 98304] sparse 0/1 (~32 nnz/row), W1:[256, 98304]
    h2  = relu(h1 @ W2.T + b2)    W2:[32, 256]
    out = h2 @ Wout.T + bout      Wout:[1, 32]  -> [2048, 1]

Strategy: data-parallel over the batch; each core handles 256 rows. Instead
of streaming the dense x (100 MB/core), the host extracts the active-feature
indices; the device gathers only the needed W1.T rows (bf16, ~4 MB/core) with
gpsimd.dma_gather and contracts them against a host-built 0/1 selection
matrix xc on the PE:

    h1.T[d, b] = sum_u W1T[U[u], d] * xc[u, b]    (U = union of the core's
                                                   active features)

dma_gather uses int16 indices (max 32767 < 98304), so the union is split into
3 windows of 32768 rows with a base-offset view of the table per window. The
SWDGE ring holds 1024 descriptors, so each window is gathered in sub-calls
(<=768 idx) spread over 4 SWDGE queues whose descriptor generation runs
concurrently. Pad slots point at the window's row 0; zeros in xc kill their
contribution. fc2/fc3 are tiny. No collectives: each core writes its own 256
outputs.
"""

import sys

sys.path.insert(0, "/opt/trn_rl_repo")

from contextlib import ExitStack

import numpy as np
import ml_dtypes

import concourse.bass as bass
import concourse.tile as tile
from concourse import bacc, mybir
from concourse.bass_utils import run_bass_kernel_spmd

f32 = mybir.dt.float32
bf16 = mybir.dt.bfloat16
i16 = mybir.dt.int16

N_CORES = 8
B = 2048
IN_DIM = 98304
H1 = 256
H2 = 32
P = 128

RPC = B // N_CORES               # 256 rows per core
NWIN = 3                         # int16 index windows over IN_DIM
WIN = 32768
CAP_W = 2816                     # gathered-index capacity per window (22 slots)
# sub-call sizes per window (each <=1024 descriptors to fit the SWDGE ring,
# multiples of 128, summing to CAP_W; first call small so the PE starts early,
# last call small so the final matmul tail is short)
SUBS_W = [
    [512, 768, 768, 768],
    [768, 768, 768, 512],
    [768, 768, 768, 512],
]
SLOTS_W = CAP_W // P             # 22
T = NWIN * SLOTS_W               # 66 k-tiles
M_T = H1 // P                    # 2 h1 partition-tiles

_CACHED = {}


def _build_program(cap_w=CAP_W, subs_w=SUBS_W):
    slots_w = cap_w // P
    t_tiles = NWIN * slots_w
    for subs in subs_w:
        assert sum(subs) == cap_w and all(s % P == 0 and s <= 1024 for s in subs)

    nc = bacc.Bacc(
        "TRN2",
        target_bir_lowering=False,
        debug=False,
        num_devices=N_CORES,
        num_swdge_queues=4,
    )

    w1t = nc.dram_tensor("w1t", [IN_DIM, H1], bf16, kind="ExternalInput")
    idxs = nc.dram_tensor("idxs", [P, NWIN, cap_w // 16], i16, kind="ExternalInput")
    xc = nc.dram_tensor("xc", [P, t_tiles, RPC], bf16, kind="ExternalInput")
    b1 = nc.dram_tensor("b1", [P, M_T], f32, kind="ExternalInput")
    w2t = nc.dram_tensor("w2t", [P, M_T, H2], f32, kind="ExternalInput")
    b2 = nc.dram_tensor("b2", [H2, 1], f32, kind="ExternalInput")
    woutt = nc.dram_tensor("woutt", [H2 + 1, 1], f32, kind="ExternalInput")
    out = nc.dram_tensor("out", [1, RPC], f32, kind="ExternalOutput")

    with tile.TileContext(nc) as tc:
        with ExitStack() as ctx:
            const = ctx.enter_context(tc.tile_pool(name="const", bufs=1))
            gp = ctx.enter_context(tc.tile_pool(name="g", bufs=1))
            actp = ctx.enter_context(tc.tile_pool(name="act", bufs=2))
            smp = ctx.enter_context(tc.tile_pool(name="small", bufs=4))
            ps1 = ctx.enter_context(tc.tile_pool(name="ps1", bufs=2, space="PSUM"))
            ps2 = ctx.enter_context(tc.tile_pool(name="ps2", bufs=2, space="PSUM"))
            ps3 = ctx.enter_context(tc.tile_pool(name="ps3", bufs=2, space="PSUM"))

            # idx load goes first and alone on the sync DMA queue: the first
            # gather depends only on it
            idx_s = const.tile([P, NWIN, cap_w // 16], i16)
            nc.sync.dma_start(idx_s[:], idxs.ap())

            # gathers: sub-calls interleaved across windows on rotating SWDGE
            # queues, so G tiles arrive spread over all three windows and the
            # final call leaves only a short matmul tail
            call_order = []
            for w in range(NWIN):
                pos = 0
                for s in subs_w[w]:
                    call_order.append((w, pos, s))
                    pos += s

            gt = gp.tile([P, t_tiles, H1], bf16, name="g", tag="g")
            for qn, (w, pos, s) in enumerate(call_order):
                s0 = w * slots_w + pos // P
                nc.gpsimd.dma_gather(
                    gt[:, s0:s0 + s // P, :],
                    w1t.ap()[w * WIN:(w + 1) * WIN, :],
                    idx_s[:, w, pos // 16:(pos + s) // 16],
                    s,
                    s,
                    H1,
                    queue_num=qn % 4,
                )

            # xc + small constants on the scalar/vector DMA queues so they
            # don't delay the idx load the gathers wait on
            xc_s = const.tile([P, t_tiles, RPC], bf16)
            for w in range(NWIN):
                sl = slice(w * slots_w, (w + 1) * slots_w)
                nc.scalar.dma_start(xc_s[:, sl], xc.ap()[:, sl])
            b1_s = const.tile([P, M_T], f32)
            nc.scalar.dma_start(b1_s[:], b1.ap())
            w2t_s = const.tile([P, M_T, H2], f32)
            nc.scalar.dma_start(w2t_s[:], w2t.ap())
            b2_s = const.tile([H2, 1], f32)
            nc.scalar.dma_start(b2_s[:], b2.ap())
            woutt_s = const.tile([H2 + 1, 1], f32)
            nc.scalar.dma_start(woutt_s[:], woutt.ap())

            # fc1: h1T[m][d, b] = sum_t G[:, t, m-slice].T @ xc[:, t, :]
            # k-tiles consumed in gather-arrival order (accumulation is
            # order-free; only the first/last need start/stop)
            t_order = [
                w * slots_w + pos // P + i
                for (w, pos, s) in call_order
                for i in range(s // P)
            ]
            assert sorted(t_order) == list(range(t_tiles))
            psum_m = [
                ps1.tile([P, RPC], f32, tag=f"ps1_{m}", name=f"ps1m{m}")
                for m in range(M_T)
            ]
            for ti, t in enumerate(t_order):
                for m in range(M_T):
                    nc.tensor.matmul(
                        psum_m[m][:],
                        gt[:, t, m * P:(m + 1) * P],
                        xc_s[:, t, :],
                        start=(ti == 0),
                        stop=(ti == t_tiles - 1),
                    )

            # bias+relu straight out of PSUM, then fc2/fc3
            acts = []
            for m in range(M_T):
                act = actp.tile([P, RPC], f32, name=f"act{m}", tag="act")
                nc.scalar.activation(
                    act[:], psum_m[m][:],
                    mybir.ActivationFunctionType.Relu,
                    bias=b1_s[:, m:m + 1],
                )
                acts.append(act)

            p2 = ps2.tile([H2, RPC], f32, name="p2", tag="p2")
            for m in range(M_T):
                nc.tensor.matmul(
                    p2[:], w2t_s[:, m, :], acts[m][:],
                    start=(m == 0), stop=(m == M_T - 1),
                )
            h2t = smp.tile([H2 + 1, RPC], f32, tag="h2", name="h2t")
            nc.scalar.activation(
                h2t[0:H2, :], p2[:],
                mybir.ActivationFunctionType.Relu,
                bias=b2_s[:],
            )
            nc.vector.memset(h2t[H2:H2 + 1, :], 1.0)

            p3 = ps3.tile([1, RPC], f32, name="p3", tag="p3")
            nc.tensor.matmul(p3[:], woutt_s[:], h2t[:], start=True, stop=True)
            ot = smp.tile([1, RPC], f32, tag="ot", name="ot")
            nc.vector.tensor_copy(ot[:], p3[:])
            nc.sync.dma_start(out.ap()[0, :], ot[:])

    nc.compile()
    return nc


def _default_subs(cap_w):
    subs = [1024] * (cap_w // 1024)
    if cap_w % 1024:
        subs.append(cap_w % 1024)
    return subs


def get_program(cap_w=CAP_W):
    key = ("nc", cap_w)
    if key not in _CACHED:
        subs_w = SUBS_W if cap_w == CAP_W else [_default_subs(cap_w)] * NWIN
        _CACHED[key] = _build_program(cap_w, subs_w)
    return _CACHED[key]


def _pack_idxs(local, cap_w):
    """[cap_w] int16 position-ordered indices -> [P, cap_w//16] SBUF layout.

    Position i is read from partition i%16, column i//16; the 16-partition
    block is replicated across all 128 partitions.
    """
    arr = local.reshape(cap_w // 16, 16).T  # [16, cols]
    return np.tile(arr, (8, 1))             # [128, cols]


def _prep_inputs(x, W1, b1, W2, b2, Wout, bout, cap_w):
    bf = ml_dtypes.bfloat16
    slots_w = cap_w // P
    t_tiles = NWIN * slots_w

    w1t_h = np.ascontiguousarray(W1.T.astype(bf))                # [98304, 256]
    b1_h = np.ascontiguousarray(b1.reshape(M_T, P).T)            # [P, M_T]
    w2t_h = np.ascontiguousarray(W2.T.reshape(M_T, P, H2).transpose(1, 0, 2))
    b2_h = np.ascontiguousarray(b2.reshape(H2, 1))
    woutt_h = np.concatenate(
        [Wout.T, bout.reshape(1, 1)], axis=0
    ).astype(np.float32)                                         # [H2+1, 1]

    rows_all, cols_all = np.nonzero(x != 0.0)

    # deal rows to cores snake-wise by nnz so per-core union sizes (and thus
    # gather descriptor counts) equalize — the slowest core sets the HW time
    nnz = np.bincount(rows_all, minlength=B)
    order = np.argsort(-nnz, kind="stable")
    core_rows = [[] for _ in range(N_CORES)]
    for i, r in enumerate(order):
        c = i % (2 * N_CORES)
        core_rows[c if c < N_CORES else 2 * N_CORES - 1 - c].append(r)
    core_rows = [np.array(rs) for rs in core_rows]
    row_of = {}  # global row -> (core, slot)
    for c in range(N_CORES):
        for k, r in enumerate(core_rows[c]):
            row_of[int(r)] = (c, k)

    slot_of = np.empty(B, dtype=np.int64)   # global row -> slot within core
    core_of = np.empty(B, dtype=np.int64)
    for r, (c, k) in row_of.items():
        core_of[r] = c
        slot_of[r] = k

    in_maps = []
    for c in range(N_CORES):
        sel = core_of[rows_all] == c
        bs = slot_of[rows_all[sel]]
        fs = cols_all[sel].astype(np.int64)
        posmap = np.full(IN_DIM, -1, dtype=np.int64)
        u_all = np.unique(fs)
        idx_h = np.zeros((P, NWIN, cap_w // 16), dtype=np.int16)
        for w in range(NWIN):
            uw = u_all[(u_all >= w * WIN) & (u_all < (w + 1) * WIN)]
            n_w = len(uw)
            if n_w > cap_w:
                raise OverflowError(n_w)
            local = np.zeros(cap_w, dtype=np.int16)
            local[:n_w] = (uw - w * WIN).astype(np.int16)
            idx_h[:, w, :] = _pack_idxs(local, cap_w)
            j = np.arange(n_w)
            posmap[uw] = (w * slots_w + j // P) * P + (j % P)
        xc_h = np.zeros((t_tiles * P, RPC), dtype=np.float32)
        xc_h[posmap[fs], bs] = 1.0
        in_maps.append({
            "w1t": w1t_h,
            "idxs": idx_h,
            "xc": np.ascontiguousarray(
                xc_h.reshape(t_tiles, P, RPC).transpose(1, 0, 2).astype(bf)
            ),
            "b1": b1_h,
            "w2t": w2t_h,
            "b2": b2_h,
            "woutt": woutt_h,
        })
    return in_maps, core_of * RPC + slot_of


def kernel(x, W1, b1, W2, b2, Wout, bout, _trace=False, _trace_kwargs=None):
    x = np.asarray(x, dtype=np.float32)
    W1 = np.asarray(W1, dtype=np.float32)
    b1 = np.asarray(b1, dtype=np.float32)
    W2 = np.asarray(W2, dtype=np.float32)
    b2 = np.asarray(b2, dtype=np.float32)
    Wout = np.asarray(Wout, dtype=np.float32)
    bout = np.asarray(bout, dtype=np.float32)

    cap_w = CAP_W
    while True:
        try:
            in_maps, out_pos = _prep_inputs(x, W1, b1, W2, b2, Wout, bout, cap_w)
            break
        except OverflowError as e:
            # denser input than expected: grow the per-window capacity
            cap_w = ((int(e.args[0]) + P - 1) // P + 1) * P

    nc = get_program(cap_w)
    res = run_bass_kernel_spmd(
        nc,
        in_maps,
        core_ids=list(range(N_CORES)),
        trace=_trace,
        **(_trace_kwargs or {}),
    )
    flat = np.concatenate(
        [res.results[c]["out"].reshape(RPC) for c in range(N_CORES)]
    )
    out = flat[out_pos].reshape(B, 1).astype(np.float32)
    if _trace:
        kernel.last_results = res
    return out


if __name__ == "__main__":
    rng = np.random.default_rng(0)
    x = (rng.random((B, IN_DIM)) < 32.0 / IN_DIM).astype(np.float32)
    W1 = rng.standard_normal((H1, IN_DIM), dtype=np.float32) / np.sqrt(IN_DIM)
    b1 = rng.standard_normal(H1, dtype=np.float32) / np.sqrt(IN_DIM)
    W2 = rng.standard_normal((H2, H1), dtype=np.float32) / np.sqrt(H1)
    b2 = rng.standard_normal(H2, dtype=np.float32) / np.sqrt(H1)
    Wout = rng.standard_normal((1, H2), dtype=np.float32) / np.sqrt(H2)
    bout = rng.standard_normal(1, dtype=np.float32) / np.sqrt(H2)
    got = kernel(x, W1, b1, W2, b2, Wout, bout)
    h1 = np.maximum(x @ W1.T + b1, 0)
    h2 = np.maximum(h1 @ W2.T + b2, 0)
    exp = h2 @ Wout.T + bout
    print("rel err:", np.abs(got - exp).max() / np.abs(exp).max())


# revision 27
# speedup vs baseline: 1.1539x; 1.0060x over previous
"""HalfKA NNUE forward pass on 8 Trainium2 NeuronCores — sparse gather version.

Network (fp32 reference):
    h1  = relu(x @ W1.T + b1)     x:[2048, 98304] sparse 0/1 (~32 nnz/row), W1:[256, 98304]
    h2  = relu(h1 @ W2.T + b2)    W2:[32, 256]
    out = h2 @ Wout.T + bout      Wout:[1, 32]  -> [2048, 1]

Strategy: data-parallel over the batch; each core handles 256 rows. Instead
of streaming the dense x (100 MB/core), the host extracts the active-feature
indices; the device gathers only the needed W1.T rows (bf16, ~4 MB/core) with
gpsimd.dma_gather and contracts them against a host-built 0/1 selection
matrix xc on the PE:

    h1.T[d, b] = sum_u W1T[U[u], d] * xc[u, b]    (U = union of the core's
                                                   active features)

dma_gather uses int16 indices (max 32767 < 98304), so the union is split into
3 windows of 32768 rows with a base-offset view of the table per window. The
SWDGE ring holds 1024 descriptors, so each window is gathered in sub-calls
(<=768 idx) spread over 4 SWDGE queues whose descriptor generation runs
concurrently. Pad slots point at the window's row 0; zeros in xc kill their
contribution. fc2/fc3 are tiny. No collectives: each core writes its own 256
outputs.
"""

import sys

sys.path.insert(0, "/opt/trn_rl_repo")

from contextlib import ExitStack

import numpy as np
import ml_dtypes

import concourse.bass as bass
import concourse.tile as tile
from concourse import bacc, mybir
from concourse.bass_utils import run_bass_kernel_spmd

f32 = mybir.dt.float32
bf16 = mybir.dt.bfloat16
i16 = mybir.dt.int16

N_CORES = 8
B = 2048
IN_DIM = 98304
H1 = 256
H2 = 32
P = 128

RPC = B // N_CORES               # 256 rows per core
NWIN = 3                         # int16 index windows over IN_DIM
WIN = 32768
CAP_W = 2816                     # gathered-index capacity per window (22 slots)
# sub-call sizes per window (each <=1024 descriptors to fit the SWDGE ring,
# multiples of 128, summing to CAP_W; first call small so the PE starts early,
# last call small so the final matmul tail is short)
SUBS_W = [
    [512, 768, 768, 768],
    [768, 768, 768, 512],
    [768, 768, 768, 512],
]
SLOTS_W = CAP_W // P             # 22
T = NWIN * SLOTS_W               # 66 k-tiles
M_T = H1 // P                    # 2 h1 partition-tiles

_CACHED = {}


def _build_program(cap_w=CAP_W, subs_w=SUBS_W):
    slots_w = cap_w // P
    t_tiles = NWIN * slots_w
    for subs in subs_w:
        assert sum(subs) == cap_w and all(s % P == 0 and s <= 1024 for s in subs)

    nc = bacc.Bacc(
        "TRN2",
        target_bir_lowering=False,
        debug=False,
        num_devices=N_CORES,
        num_swdge_queues=4,
    )

    w1t = nc.dram_tensor("w1t", [IN_DIM, H1], bf16, kind="ExternalInput")
    idxs = nc.dram_tensor("idxs", [P, NWIN, cap_w // 16], i16, kind="ExternalInput")
    xc = nc.dram_tensor("xc", [P, t_tiles, RPC], bf16, kind="ExternalInput")
    b1 = nc.dram_tensor("b1", [P, M_T], f32, kind="ExternalInput")
    w2t = nc.dram_tensor("w2t", [P, M_T, H2], f32, kind="ExternalInput")
    b2 = nc.dram_tensor("b2", [H2, 1], f32, kind="ExternalInput")
    woutt = nc.dram_tensor("woutt", [H2 + 1, 1], f32, kind="ExternalInput")
    out = nc.dram_tensor("out", [1, RPC], f32, kind="ExternalOutput")

    with tile.TileContext(nc) as tc:
        with ExitStack() as ctx:
            const = ctx.enter_context(tc.tile_pool(name="const", bufs=1))
            gp = ctx.enter_context(tc.tile_pool(name="g", bufs=1))
            actp = ctx.enter_context(tc.tile_pool(name="act", bufs=2))
            smp = ctx.enter_context(tc.tile_pool(name="small", bufs=4))
            ps1 = ctx.enter_context(tc.tile_pool(name="ps1", bufs=2, space="PSUM"))
            ps2 = ctx.enter_context(tc.tile_pool(name="ps2", bufs=2, space="PSUM"))
            ps3 = ctx.enter_context(tc.tile_pool(name="ps3", bufs=2, space="PSUM"))

            # idx load goes first and alone on the sync DMA queue: the first
            # gather depends only on it
            idx_s = const.tile([P, NWIN, cap_w // 16], i16)
            nc.sync.dma_start(idx_s[:], idxs.ap())

            # gathers: sub-calls interleaved across windows on rotating SWDGE
            # queues, so G tiles arrive spread over all three windows and the
            # final call leaves only a short matmul tail
            call_order = []
            for w in range(NWIN):
                pos = 0
                for s in subs_w[w]:
                    call_order.append((w, pos, s))
                    pos += s

            gt = gp.tile([P, t_tiles, H1], bf16, name="g", tag="g")
            for qn, (w, pos, s) in enumerate(call_order):
                s0 = w * slots_w + pos // P
                nc.gpsimd.dma_gather(
                    gt[:, s0:s0 + s // P, :],
                    w1t.ap()[w * WIN:(w + 1) * WIN, :],
                    idx_s[:, w, pos // 16:(pos + s) // 16],
                    s,
                    s,
                    H1,
                    queue_num=qn % 4,
                )

            # xc + small constants on the scalar/vector DMA queues so they
            # don't delay the idx load the gathers wait on
            xc_s = const.tile([P, t_tiles, RPC], bf16)
            for w in range(NWIN):
                sl = slice(w * slots_w, (w + 1) * slots_w)
                nc.scalar.dma_start(xc_s[:, sl], xc.ap()[:, sl])
            b1_s = const.tile([P, M_T], f32)
            nc.scalar.dma_start(b1_s[:], b1.ap())
            w2t_s = const.tile([P, M_T, H2], f32)
            nc.scalar.dma_start(w2t_s[:], w2t.ap())
            b2_s = const.tile([H2, 1], f32)
            nc.scalar.dma_start(b2_s[:], b2.ap())
            woutt_s = const.tile([H2 + 1, 1], f32)
            nc.scalar.dma_start(woutt_s[:], woutt.ap())

            # fc1: h1T[m][d, b] = sum_t G[:, t, m-slice].T @ xc[:, t, :]
            # k-tiles consumed in gather-arrival order (accumulation is
            # order-free; only the first/last need start/stop)
            t_order = [
                w * slots_w + pos // P + i
                for (w, pos, s) in call_order
                for i in range(s // P)
            ]
            assert sorted(t_order) == list(range(t_tiles))
            psum_m = [
                ps1.tile([P, RPC], f32, tag=f"ps1_{m}", name=f"ps1m{m}")
                for m in range(M_T)
            ]
            for ti, t in enumerate(t_order):
                for m in range(M_T):
                    nc.tensor.matmul(
                        psum_m[m][:],
                        gt[:, t, m * P:(m + 1) * P],
                        xc_s[:, t, :],
                        start=(ti == 0),
                        stop=(ti == t_tiles - 1),
                    )

            # bias+relu straight out of PSUM, then fc2/fc3
            acts = []
            for m in range(M_T):
                act = actp.tile([P, RPC], f32, name=f"act{m}", tag="act")
                nc.scalar.activation(
                    act[:], psum_m[m][:],
                    mybir.ActivationFunctionType.Relu,
                    bias=b1_s[:, m:m + 1],
                )
                acts.append(act)

            p2 = ps2.tile([H2, RPC], f32, name="p2", tag="p2")
            for m in range(M_T):
                nc.tensor.matmul(
                    p2[:], w2t_s[:, m, :], acts[m][:],
                    start=(m == 0), stop=(m == M_T - 1),
                )
            h2t = smp.tile([H2 + 1, RPC], f32, tag="h2", name="h2t")
            nc.scalar.activation(
                h2t[0:H2, :], p2[:],
                mybir.ActivationFunctionType.Relu,
                bias=b2_s[:],
            )
            nc.vector.memset(h2t[H2:H2 + 1, :], 1.0)

            p3 = ps3.tile([1, RPC], f32, name="p3", tag="p3")
            nc.tensor.matmul(p3[:], woutt_s[:], h2t[:], start=True, stop=True)
            ot = smp.tile([1, RPC], f32, tag="ot", name="ot")
            nc.vector.tensor_copy(ot[:], p3[:])
            nc.sync.dma_start(out.ap()[0, :], ot[:])

    nc.compile()
    return nc


def _default_subs(cap_w):
    subs = [1024] * (cap_w // 1024)
    if cap_w % 1024:
        subs.append(cap_w % 1024)
    return subs


def get_program(cap_w=CAP_W):
    key = ("nc", cap_w)
    if key not in _CACHED:
        subs_w = SUBS_W if cap_w == CAP_W else [_default_subs(cap_w)] * NWIN
        _CACHED[key] = _build_program(cap_w, subs_w)
    return _CACHED[key]


def _pack_idxs(local, cap_w):
    """[cap_w] int16 position-ordered indices -> [P, cap_w//16] SBUF layout.

    Position i is read from partition i%16, column i//16; the 16-partition
    block is replicated across all 128 partitions.
    """
    arr = local.reshape(cap_w // 16, 16).T  # [16, cols]
    return np.tile(arr, (8, 1))             # [128, cols]


def _prep_inputs(x, W1, b1, W2, b2, Wout, bout, cap_w):
    bf = ml_dtypes.bfloat16
    slots_w = cap_w // P
    t_tiles = NWIN * slots_w

    w1t_h = np.ascontiguousarray(W1.T.astype(bf))                # [98304, 256]
    b1_h = np.ascontiguousarray(b1.reshape(M_T, P).T)            # [P, M_T]
    w2t_h = np.ascontiguousarray(W2.T.reshape(M_T, P, H2).transpose(1, 0, 2))
    b2_h = np.ascontiguousarray(b2.reshape(H2, 1))
    woutt_h = np.concatenate(
        [Wout.T, bout.reshape(1, 1)], axis=0
    ).astype(np.float32)                                         # [H2+1, 1]

    rows_all, cols_all = np.nonzero(x != 0.0)

    # deal rows to cores snake-wise by nnz so per-core union sizes (and thus
    # gather descriptor counts) equalize — the slowest core sets the HW time
    nnz = np.bincount(rows_all, minlength=B)
    order = np.argsort(-nnz, kind="stable")
    core_rows = [[] for _ in range(N_CORES)]
    for i, r in enumerate(order):
        c = i % (2 * N_CORES)
        core_rows[c if c < N_CORES else 2 * N_CORES - 1 - c].append(r)
    core_rows = [np.array(rs) for rs in core_rows]
    row_of = {}  # global row -> (core, slot)
    for c in range(N_CORES):
        for k, r in enumerate(core_rows[c]):
            row_of[int(r)] = (c, k)

    slot_of = np.empty(B, dtype=np.int64)   # global row -> slot within core
    core_of = np.empty(B, dtype=np.int64)
    for r, (c, k) in row_of.items():
        core_of[r] = c
        slot_of[r] = k

    in_maps = []
    for c in range(N_CORES):
        sel = core_of[rows_all] == c
        bs = slot_of[rows_all[sel]]
        fs = cols_all[sel].astype(np.int64)
        posmap = np.full(IN_DIM, -1, dtype=np.int64)
        u_all = np.unique(fs)
        idx_h = np.zeros((P, NWIN, cap_w // 16), dtype=np.int16)
        for w in range(NWIN):
            uw = u_all[(u_all >= w * WIN) & (u_all < (w + 1) * WIN)]
            n_w = len(uw)
            if n_w > cap_w:
                raise OverflowError(n_w)
            local = np.zeros(cap_w, dtype=np.int16)
            local[:n_w] = (uw - w * WIN).astype(np.int16)
            idx_h[:, w, :] = _pack_idxs(local, cap_w)
            j = np.arange(n_w)
            posmap[uw] = (w * slots_w + j // P) * P + (j % P)
        xc_h = np.zeros((t_tiles * P, RPC), dtype=np.float32)
        xc_h[posmap[fs], bs] = 1.0
        in_maps.append({
            "w1t": w1t_h,
            "idxs": idx_h,
            "xc": np.ascontiguousarray(
                xc_h.reshape(t_tiles, P, RPC).transpose(1, 0, 2).astype(bf)
            ),
            "b1": b1_h,
            "w2t": w2t_h,
            "b2": b2_h,
            "woutt": woutt_h,
        })
    return in_maps, core_of * RPC + slot_of


def kernel(x, W1, b1, W2, b2, Wout, bout, _trace=False, _trace_kwargs=None):
    x = np.asarray(x, dtype=np.float32)
    W1 = np.asarray(W1, dtype=np.float32)
    b1 = np.asarray(b1, dtype=np.float32)
    W2 = np.asarray(W2, dtype=np.float32)
    b2 = np.asarray(b2, dtype=np.float32)
    Wout = np.asarray(Wout, dtype=np.float32)
    bout = np.asarray(bout, dtype=np.float32)

    cap_w = CAP_W
    while True:
        try:
            in_maps, out_pos = _prep_inputs(x, W1, b1, W2, b2, Wout, bout, cap_w)
            break
        except OverflowError as e:
            # denser input than expected: grow the per-window capacity
            cap_w = ((int(e.args[0]) + P - 1) // P + 1) * P

    nc = get_program(cap_w)
    res = run_bass_kernel_spmd(
        nc,
        in_maps,
        core_ids=list(range(N_CORES)),
        trace=_trace,
        **(_trace_kwargs or {}),
    )
    flat = np.concatenate(
        [res.results[c]["out"].reshape(RPC) for c in range(N_CORES)]
    )
    out = flat[out_pos].reshape(B, 1).astype(np.float32)
    if _trace:
        kernel.last_results = res
    return out


if __name__ == "__main__":
    rng = np.random.default_rng(0)
    x = (rng.random((B, IN_DIM)) < 32.0 / IN_DIM).astype(np.float32)
    W1 = rng.standard_normal((H1, IN_DIM), dtype=np.float32) / np.sqrt(IN_DIM)
    b1 = rng.standard_normal(H1, dtype=np.float32) / np.sqrt(IN_DIM)
    W2 = rng.standard_normal((H2, H1), dtype=np.float32) / np.sqrt(H1)
    b2 = rng.standard_normal(H2, dtype=np.float32) / np.sqrt(H1)
    Wout = rng.standard_normal((1, H2), dtype=np.float32) / np.sqrt(H2)
    bout = rng.standard_normal(1, dtype=np.float32) / np.sqrt(H2)
    got = kernel(x, W1, b1, W2, b2, Wout, bout)
    h1 = np.maximum(x @ W1.T + b1, 0)
    h2 = np.maximum(h1 @ W2.T + b2, 0)
    exp = h2 @ Wout.T + bout
    print("rel err:", np.abs(got - exp).max() / np.abs(exp).max())
